# revision 33
# baseline (speedup 1.0000x reference)
import numpy as np

# nn_DigitCaps dynamic routing on TRN2 Bass: B=512, N=1152, O=10, D=16, I=8.
#
# Math: with b_ij zero-init and updated as b_ij += u_hat * sum_d(v), the
# routing logits stay rank-1: b = u_hat[b,n,o,d] * T[b,o] with T accumulating
# sum_d(v) across iterations.  Each iteration needs, per (b,o,d):
#   den = sum_n exp(u*T - m),  num = sum_n u*exp(u*T - m),  s = num/den,
# then v = squash(s), T += sum_d v.  Routing is independent per (b,o) pair.
#
# Precision: |T| reaches ~8.6 by iter 3 and a few (b,o) softmaxes saturate,
# so logit noise is amplified ~|T|*|u| ~ 200x; bf16/fp16 anywhere on the
# u/e path fails the 2e-2 gate (measured 0.03-0.33).  Everything stays fp32.
#
# Sharding: 8 cores = 4 batch-groups x 2 o-groups (no collectives).  Each core
# owns 128 batch samples x 5 output capsules, processed as 5 chunks (one per
# o) with SBUF partition dim = the 128 batch samples.
#
# u_hat per chunk via TensorE: stationary = x packed as [(16n x 8i)=128,
# 128b] blocks, moving = W packed block-diagonally [(16n x 8i)=128,
# (16n x 16d)=256].  The block-diagonal W (16x inflation) is built ON-CHIP
# from the dense pack by 16 small SBUF->SBUF DMAs per o (engine copies can't
# start at unaligned partitions), so HBM/input traffic per core is ~7 MB
# instead of ~55 MB.
#
# Engine split per (o, iter): ACT does exp per d-slice with accum_out giving
# den for free (the old separate DVE tensor_reduce was ~190us/core); DVE and
# GPSIMD split the num = sum_n u*e scalar_tensor_tensor accum 8/8.  u is
# evacuated PSUM->SBUF in d-halves (3 rotating half-buffers) so chunk o+1's
# matmuls+evac overlap chunk o's routing.
#
# Stability: m = |T| * max_n(||W[n,o,d,:]|| * ||x[b,n,:]||) - SHIFT upper-
# bounds max_n(u*T) via Cauchy-Schwarz (host precomputes the norm products),
# so no on-device max/min passes are needed.

B, N, O, I, D = 512, 1152, 10, 8, 16
N_CORES = 8
BG, OG = 4, 2            # core grid: batch-groups x o-groups
BL = B // BG             # 128 batch per core
OL = O // OG             # 5 o-capsules per core
NBLK = N // 16           # 72 blocks of 16 input capsules
GB = 6                   # matmul blocks per PSUM evacuation group
NGRP = NBLK // GB        # 12 evacuation groups per o-chunk
SHIFT = 60.0
N_DVE_D = 8              # num-pass d-slices on DVE; rest go to GPSIMD

_STATE = {}


def _build_bass(reps=1):
    import concourse.bass as bass
    import concourse.tile as tile
    from concourse import bacc, mybir

    f32 = mybir.dt.float32
    bf16 = mybir.dt.bfloat16
    OP = mybir.AluOpType
    AF = mybir.ActivationFunctionType

    nc = bacc.Bacc("TRN2", target_bir_lowering=False, debug=False,
                   num_devices=N_CORES)

    xs_d = nc.dram_tensor("xs", [128, NBLK, BL], f32, kind="ExternalInput").ap()
    wd_d = nc.dram_tensor("wd", [128, OL, NBLK, 16], f32,
                          kind="ExternalInput").ap()
    kcs_d = nc.dram_tensor("kcs", [OL, BL, 1], f32, kind="ExternalInput").ap()
    v_d = nc.dram_tensor("v", [OL, BL, 16], f32, kind="ExternalOutput").ap()

    with tile.TileContext(nc) as tc:
        from contextlib import ExitStack
        ctx = ExitStack()
        u_pool = ctx.enter_context(tc.tile_pool(name="u", bufs=3))
        wbd_pool = ctx.enter_context(tc.tile_pool(name="wbd", bufs=1))
        wd_pool = ctx.enter_context(tc.tile_pool(name="wd", bufs=1))
        xs_pool = ctx.enter_context(tc.tile_pool(name="xs", bufs=2))
        e_pool = ctx.enter_context(tc.tile_pool(name="e", bufs=2))
        st_pool = ctx.enter_context(tc.tile_pool(name="st", bufs=2))
        scr_pool = ctx.enter_context(tc.tile_pool(name="scr", bufs=1))
        pu_pool = ctx.enter_context(tc.tile_pool(name="pu", bufs=2, space="PSUM"))
        ps1_pool = ctx.enter_context(tc.tile_pool(name="ps1", bufs=1, space="PSUM"))

        u32 = mybir.dt.uint32
        shift_t = st_pool.tile([BL, 1], f32, name="shift")
        nc.vector.memset(shift_t[:], SHIFT)
        ones_t = st_pool.tile([BL, 16], f32, name="ones")
        nc.vector.memset(ones_t[:], 1.0)
        magic_u = st_pool.tile([BL, 1], u32, name="magic")
        nc.vector.memset(magic_u[:], 0x5f3759df)

        # block-diagonal moving operand, zeroed once; diagonal blocks are
        # overwritten per o (same positions every o, zeros stay zero).
        wbd_t = wbd_pool.tile([128, NBLK, 256], f32, name="wbd")
        half = NBLK * 256 // 2
        nc.vector.memset(wbd_t[:].rearrange("p a b -> p (a b)")[:, :half], 0.0)
        nc.scalar.memzero(wbd_t[:].rearrange("p a b -> p (a b)")[:, half:])

        # persistent scratch for STT main outputs (value never read)
        scr_v = scr_pool.tile([BL, N], bf16, name="scrv")

        for oi in range(reps * OL):
            o = oi % OL
            # dense W for this o (feeds the diag build + iter-1 mean matmul)
            wd_t = wd_pool.tile([128, NBLK, 16], f32, name=f"wd{oi}", tag="wd")
            nc.gpsimd.dma_start(wd_t[:], wd_d[:, o])

            # ---- build block-diagonal W (SBUF->SBUF DMA; engine ops can't
            # start at unaligned partitions) ----
            if oi == 0:
                bqueues = [nc.sync, nc.gpsimd, nc.scalar]
            else:
                bqueues = [nc.sync, nc.gpsimd]
            for nn in range(16):
                dst = wbd_t[nn * 8:(nn + 1) * 8, :, nn * 16:(nn + 1) * 16]
                src = wd_t[nn * 8:(nn + 1) * 8, :, :]
                bqueues[nn % len(bqueues)].dma_start(dst, src)

            kcs_t = st_pool.tile([BL, 1], f32, name=f"kcs{oi}", tag="kcs")
            nc.gpsimd.dma_start(kcs_t[:], kcs_d[o])

            # u in two d-halves (separate tiles from a 3-deep pool so the
            # next chunk's evacuation overlaps this chunk's routing)
            uh = [u_pool.tile([128, 8, N], f32, name=f"u{oi}_{h}", tag="u")
                  for h in range(2)]
            ps1 = ps1_pool.tile([128, 16], f32, name=f"s1p{oi}", tag="s1p")

            # ---- phase 1: u_hat matmuls + PSUM->SBUF evacuation ----
            # (priority-boosted ~one chunk so the scheduler interleaves this
            # with the previous chunk's routing instead of queueing behind it)
            ctx_p = tc.high_priority(offset=800) if oi > 0 else None
            if ctx_p is not None:
                ctx_p.__enter__()
            for g in range(NGRP):
                pu = pu_pool.tile([128, GB, 256], f32, name=f"pu{oi}_{g}",
                                  tag="pu")
                xs_t = xs_pool.tile([128, GB, BL], f32, name=f"x{oi}_{g}",
                                    tag="xs")
                nc.gpsimd.dma_start(xs_t[:], xs_d[:, g * GB:(g + 1) * GB, :])
                for j in range(GB):
                    blk = g * GB + j
                    nc.tensor.matmul(pu[:, j, :], lhsT=xs_t[:, j, :],
                                     rhs=wbd_t[:, blk, :],
                                     start=True, stop=True,
                                     skip_group_check=True)
                    nc.tensor.matmul(ps1[:], lhsT=xs_t[:, j, :],
                                     rhs=wd_t[:, blk, :],
                                     start=(blk == 0), stop=(blk == NBLK - 1),
                                     skip_group_check=True)
                # evacuate: [128, (j, nn16, d16)] -> u[128, d, g*96+(j,nn)]
                for h in range(2):
                    src = pu[:].rearrange(
                        "p j (nn d) -> p d j nn", nn=16, d=16
                    )[:, h * 8:(h + 1) * 8]
                    dst = uh[h][:, :, g * 96:(g + 1) * 96].rearrange(
                        "p d (j nn) -> p d j nn", j=GB, nn=16)
                    if (2 * g + h) % 5 < 2:
                        nc.scalar.copy(dst, src)
                    else:
                        nc.vector.tensor_copy(dst, src)

            if ctx_p is not None:
                ctx_p.__exit__(None, None, None)

            # ---- iteration 1 (uniform softmax): s = mean_n u ----
            s_t = st_pool.tile([BL, 16], f32, name=f"s{oi}", tag="s")
            nc.scalar.mul(s_t[:], ps1[:], 1.0 / N)

            sq2 = st_pool.tile([BL, 16], f32, name=f"sq2_{oi}", tag="sq2")
            T_t = st_pool.tile([BL, 1], f32, name=f"T{oi}", tag="T")
            vout = st_pool.tile([BL, 16], f32, name=f"vo{oi}", tag="vo")

            def squash_T(tag, accum_into_T, want_T=True):
                # f = sqrt(sn)/(1+sn); vout = f*s; Tdelta = sum_d vout
                # sqrt via fast-inverse-sqrt (bit trick + 3 Newton steps),
                # all on DVE: the ACT Ln/Exp route forces a ~1.3-2.7us table
                # reload per call (Ln and Exp land in different table sets).
                sn = st_pool.tile([BL, 1], f32, name=f"sn{tag}", tag="sn")
                nc.vector.scalar_tensor_tensor(
                    sq2[:], s_t[:], 1.0, s_t[:], op0=OP.mult, op1=OP.mult,
                    accum_out=sn[:])
                y_t = st_pool.tile([BL, 1], f32, name=f"y{tag}", tag="y")
                t2 = st_pool.tile([BL, 1], f32, name=f"t2{tag}", tag="t2")
                # y0 bits = MAGIC - (sn_bits >> 1)
                nc.vector.tensor_scalar(
                    t2[:].bitcast(u32), sn[:].bitcast(u32), 1, None,
                    op0=OP.logical_shift_right)
                nc.vector.tensor_tensor(
                    y_t[:].bitcast(u32), magic_u[:], t2[:].bitcast(u32),
                    op=OP.subtract)
                for _ in range(2):
                    # y <- y * (1.5 - 0.5*sn*y^2), 3 fused DVE ops
                    nc.vector.tensor_mul(t2[:], y_t[:], y_t[:])
                    nc.vector.scalar_tensor_tensor(
                        t2[:], sn[:], -0.5, t2[:], op0=OP.mult, op1=OP.mult)
                    nc.vector.scalar_tensor_tensor(
                        y_t[:], t2[:], 1.5, y_t[:], op0=OP.add, op1=OP.mult)
                snp = st_pool.tile([BL, 1], f32, name=f"sp{tag}", tag="snp")
                nc.vector.tensor_scalar_add(snp[:], sn[:], 1.0)
                rsn = st_pool.tile([BL, 1], f32, name=f"rs{tag}", tag="rsn")
                nc.vector.reciprocal(rsn[:], snp[:])
                f_t = st_pool.tile([BL, 1], f32, name=f"f{tag}", tag="f")
                nc.vector.scalar_tensor_tensor(
                    f_t[:], sn[:], rsn[:], y_t[:], op0=OP.mult, op1=OP.mult)
                if not want_T:
                    nc.vector.scalar_tensor_tensor(
                        vout[:], s_t[:], f_t[:], ones_t[:],
                        op0=OP.mult, op1=OP.mult)
                    return
                dT = st_pool.tile([BL, 1], f32, name=f"dT{tag}", tag="dT")
                nc.vector.scalar_tensor_tensor(
                    vout[:], s_t[:], f_t[:], ones_t[:],
                    op0=OP.mult, op1=OP.mult, accum_out=dT[:])
                if accum_into_T:
                    nc.vector.tensor_add(T_t[:], T_t[:], dT[:])
                else:
                    nc.vector.tensor_copy(T_t[:], dT[:])

            squash_T(f"{oi}i1", accum_into_T=False)

            den_t = st_pool.tile([BL, 16], f32, name=f"den{oi}", tag="den")
            num_t = st_pool.tile([BL, 16], f32, name=f"num{oi}", tag="num")

            for it in (2, 3):
                absT = st_pool.tile([BL, 1], f32, name=f"aT{oi}_{it}", tag="aT")
                nc.vector.scalar_tensor_tensor(
                    absT[:], T_t[:], -1.0, T_t[:], op0=OP.mult, op1=OP.max)
                negm = st_pool.tile([BL, 1], f32, name=f"nm{oi}_{it}", tag="nm")
                nc.vector.scalar_tensor_tensor(
                    negm[:], absT[:], kcs_t[:], shift_t[:],
                    op0=OP.mult, op1=OP.add)
                for d in range(D):
                    u_sl = uh[d // 8][:, d % 8, :]
                    e_t = e_pool.tile([128, N], f32, name=f"e{oi}_{it}_{d}",
                                      tag="e")
                    nc.scalar.activation(e_t[:], u_sl, AF.Exp,
                                         bias=negm[:], scale=T_t[:],
                                         accum_out=den_t[:, d:d + 1])
                    nc.vector.scalar_tensor_tensor(
                        out=scr_v[:], in0=e_t[:], scalar=1.0,
                        in1=u_sl, op0=OP.mult, op1=OP.mult,
                        accum_out=num_t[:, d:d + 1])
                rden = st_pool.tile([BL, 16], f32, name=f"rd{oi}_{it}", tag="rd")
                nc.vector.reciprocal(rden[:], den_t[:])
                nc.vector.tensor_mul(s_t[:], num_t[:], rden[:])
                squash_T(f"{oi}i{it}", accum_into_T=(it == 2), want_T=(it == 2))

            nc.gpsimd.dma_start(v_d[o], vout[:])
        ctx.close()

    nc.compile()
    return nc


def _pack_w(W):
    # W: [N, O, D, I] fp32 -> per o-group: dense pack [(nn,i), o, blk, d]
    # plus the Cauchy-Schwarz bound factors.
    packs = []
    for og in range(OG):
        Wl = W[:, og * OL:(og + 1) * OL]                     # [N, 5, D, I]
        Wt = Wl.reshape(NBLK, 16, OL, D, I)                  # [blk, nn, o, d, i]
        wd = np.ascontiguousarray(
            Wt.transpose(1, 4, 2, 0, 3).reshape(128, OL, NBLK, 16)
        ).astype(np.float32)
        wmax = np.sqrt((Wl ** 2).sum(-1)).max(axis=(0, 2))   # [5]
        packs.append((wd, wmax.astype(np.float32)))
    return packs


def _get_state(W):
    key = (W.shape, hash(W[::131, 0, 0, 0].tobytes()))
    st = _STATE.get("st")
    if st is not None and st["key"] == key:
        return st
    nc = _STATE.get("nc")
    if nc is None:
        nc = _build_bass()
        _STATE["nc"] = nc
    st = {"key": key, "nc": nc, "wpacks": _pack_w(W)}
    _STATE["st"] = st
    return st


def _core_inputs(st, x, xnorm):
    # returns in_maps for the 8 cores; core index = og * BG + bg
    in_maps = []
    for og in range(OG):
        wd, wmax = st["wpacks"][og]
        for bg in range(BG):
            b0 = bg * BL
            xc = x[b0:b0 + BL]                               # [128, N, I]
            xs = np.ascontiguousarray(
                xc.transpose(1, 2, 0).reshape(NBLK, 128, BL)
                .transpose(1, 0, 2))                         # [p, blk, b]
            kcs = -(wmax[:, None] * xnorm[None, b0:b0 + BL]) * 1.001  # [5,128]
            in_maps.append({
                "xs": xs, "wd": wd,
                "kcs": np.ascontiguousarray(
                    kcs[:, :, None].astype(np.float32)),
            })
    return in_maps


class _Runner:
    """Cached PJRT executor for the compiled Bass program (modeled on
    bass2jax.run_bass_via_pjrt, but holding the jitted callable so repeat
    calls don't re-trace)."""

    def __init__(self, nc):
        import jax
        import numpy as _np
        from jax.sharding import Mesh, PartitionSpec
        from jax.experimental.shard_map import shard_map
        from concourse import mybir
        from concourse.bass2jax import (_bass_exec_p, install_neuronx_cc_hook,
                                        partition_id_tensor)

        install_neuronx_cc_hook()
        self.jax = jax
        part_name = (nc.partition_id_tensor.name
                     if nc.partition_id_tensor else None)
        in_names, out_names, out_avals, zero_outs = [], [], [], []
        for alloc in nc.m.functions[0].allocations:
            if not isinstance(alloc, mybir.MemoryLocationSet):
                continue
            name = alloc.memorylocations[0].name
            if alloc.kind == "ExternalInput":
                if name == part_name:
                    continue
                in_names.append(name)
            elif alloc.kind == "ExternalOutput":
                shape = tuple(alloc.tensor_shape)
                dtype = mybir.dt.np(alloc.dtype)
                out_names.append(name)
                out_avals.append(jax.core.ShapedArray(shape, dtype))
                zero_outs.append(_np.zeros(shape, dtype))
        self.in_names, self.out_names = in_names, out_names
        self.out_avals, self.zero_outs = out_avals, zero_outs
        n_params, n_outs = len(in_names), len(out_names)
        all_names = in_names + out_names
        if part_name is not None:
            all_names = all_names + [part_name]

        def _body(*args):
            ins = list(args[:n_params])
            outs = list(args[n_params:])
            pid = [partition_id_tensor()] if part_name is not None else []
            outs = list(_bass_exec_p.bind(
                *ins, *outs, *pid,
                out_avals=tuple(out_avals),
                in_names=tuple(all_names),
                out_names=tuple(out_names),
                lowering_input_output_aliases=(),
                sim_require_finite=True, sim_require_nnan=True,
                nc=nc))
            return tuple(outs)

        devices = jax.devices()[:N_CORES]
        self.mesh = Mesh(_np.asarray(devices), ("core",))
        in_specs = (PartitionSpec("core"),) * (n_params + n_outs)
        out_specs = (PartitionSpec("core"),) * n_outs
        self.fn = jax.jit(
            shard_map(_body, mesh=self.mesh,
                      in_specs=in_specs, out_specs=out_specs,
                      check_rep=False),
            donate_argnums=tuple(range(n_params, n_params + n_outs)),
            keep_unused=True)

    def concat_inputs(self, in_maps):
        import numpy as _np
        return [_np.concatenate([m[name] for m in in_maps], axis=0)
                for name in self.in_names]

    def zeros(self):
        import numpy as _np
        return [_np.zeros((N_CORES * z.shape[0], *z.shape[1:]), z.dtype)
                for z in self.zero_outs]

    def __call__(self, concat_in):
        return self.fn(*concat_in, *self.zeros())


def _run(st, in_maps):
    runner = st.get("runner")
    if runner is None:
        runner = _Runner(st["nc"])
        st["runner"] = runner
    out_arrs = runner(runner.concat_inputs(in_maps))
    av = runner.out_avals[0]
    import numpy as _np
    v_all = _np.asarray(out_arrs[0]).reshape(N_CORES, *av.shape)
    return v_all


def kernel(x: np.ndarray, W: np.ndarray) -> np.ndarray:
    x = np.ascontiguousarray(x, dtype=np.float32)
    W = np.ascontiguousarray(W, dtype=np.float32)
    st = _get_state(W)
    xnorm = np.sqrt((x ** 2).sum(-1)).max(axis=1).astype(np.float32)  # [B]
    in_maps = _core_inputs(st, x, xnorm)
    v_all = _run(st, in_maps)
    out = np.empty((B, O, D), np.float32)
    ci = 0
    for og in range(OG):
        for bg in range(BG):
            v = v_all[ci]                                     # [5, 128, 16]
            out[bg * BL:(bg + 1) * BL, og * OL:(og + 1) * OL, :] = \
                v.transpose(1, 0, 2)
            ci += 1
    return out


def hw_exec_ns(x: np.ndarray, W: np.ndarray, reps: int = 4,
               chain_len: int = 16) -> int:
    """Device execution time per kernel body, measured by differencing two
    NEFFs: one with the body once, one with it repeated `reps` times.  Both
    are dispatched as identical async call-chains, so host/axon dispatch
    overhead cancels exactly in the subtraction."""
    import time
    import jax
    x = np.ascontiguousarray(x, dtype=np.float32)
    W = np.ascontiguousarray(W, dtype=np.float32)
    st = _get_state(W)
    xnorm = np.sqrt((x ** 2).sum(-1)).max(axis=1).astype(np.float32)
    in_maps = _core_inputs(st, x, xnorm)
    runner = st.get("runner")
    if runner is None:
        runner = _Runner(st["nc"])
        st["runner"] = runner
    nc_r = _STATE.get("nc_rep")
    if nc_r is None:
        nc_r = _build_bass(reps=reps)
        _STATE["nc_rep"] = nc_r
    runner_r = st.get("runner_rep")
    if runner_r is None:
        runner_r = _Runner(nc_r)
        st["runner_rep"] = runner_r

    from jax.sharding import NamedSharding, PartitionSpec
    sh = NamedSharding(runner.mesh, PartitionSpec("core"))
    dev_in = [jax.device_put(a, sh) for a in runner.concat_inputs(in_maps)]

    def chain(fn, n):
        outs = [jax.device_put(z, sh) for z in runner.zeros()]
        jax.block_until_ready(outs)
        t0 = time.perf_counter()
        for _ in range(n):
            outs = fn(*dev_in, *outs)
        jax.block_until_ready(outs)
        return time.perf_counter() - t0

    chain(runner.fn, 2)       # warm + compile
    chain(runner_r.fn, 2)
    t1 = min(chain(runner.fn, chain_len) for _ in range(6))
    tr = min(chain(runner_r.fn, chain_len) for _ in range(6))
    return int((tr - t1) / (chain_len * (reps - 1)) * 1e9)


# revision 35
# speedup vs baseline: 1.0482x; 1.0482x over previous
import numpy as np

# nn_DigitCaps dynamic routing on TRN2 Bass: B=512, N=1152, O=10, D=16, I=8.
#
# Math: with b_ij zero-init and updated as b_ij += u_hat * sum_d(v), the
# routing logits stay rank-1: b = u_hat[b,n,o,d] * T[b,o] with T accumulating
# sum_d(v) across iterations.  Each iteration needs, per (b,o,d):
#   den = sum_n exp(u*T - m),  num = sum_n u*exp(u*T - m),  s = num/den,
# then v = squash(s), T += sum_d v.  Routing is independent per (b,o) pair.
#
# Precision: |T| reaches ~8.6 by iter 3 and a few (b,o) softmaxes saturate,
# so logit noise is amplified ~|T|*|u| ~ 200x; bf16/fp16 anywhere on the
# u/e path fails the 2e-2 gate (measured 0.03-0.33).  Everything stays fp32.
#
# Sharding: 8 cores = 4 batch-groups x 2 o-groups (no collectives).  Each core
# owns 128 batch samples x 5 output capsules, processed as 5 chunks (one per
# o) with SBUF partition dim = the 128 batch samples.
#
# u_hat per chunk via TensorE: stationary = x packed as [(16n x 8i)=128,
# 128b] blocks, moving = W packed block-diagonally [(16n x 8i)=128,
# (16n x 16d)=256].  The block-diagonal W (16x inflation) is built ON-CHIP
# from the dense pack by 16 small SBUF->SBUF DMAs per o (engine copies can't
# start at unaligned partitions), so HBM/input traffic per core is ~7 MB
# instead of ~55 MB.
#
# Engine split per (o, iter): ACT does exp per d-slice with accum_out giving
# den for free (the old separate DVE tensor_reduce was ~190us/core); DVE and
# GPSIMD split the num = sum_n u*e scalar_tensor_tensor accum 8/8.  u is
# evacuated PSUM->SBUF in d-halves (3 rotating half-buffers) so chunk o+1's
# matmuls+evac overlap chunk o's routing.
#
# Stability: m = |T| * max_n(||W[n,o,d,:]|| * ||x[b,n,:]||) - SHIFT upper-
# bounds max_n(u*T) via Cauchy-Schwarz (host precomputes the norm products),
# so no on-device max/min passes are needed.

B, N, O, I, D = 512, 1152, 10, 8, 16
N_CORES = 8
BG, OG = 4, 2            # core grid: batch-groups x o-groups
BL = B // BG             # 128 batch per core
OL = O // OG             # 5 o-capsules per core
NBLK = N // 16           # 72 blocks of 16 input capsules
GB = 6                   # matmul blocks per PSUM evacuation group
NGRP = NBLK // GB        # 12 evacuation groups per o-chunk
SHIFT = 60.0
N_DVE_D = 8              # num-pass d-slices on DVE; rest go to GPSIMD

_STATE = {}


def _build_bass(reps=1):
    import concourse.bass as bass
    import concourse.tile as tile
    from concourse import bacc, mybir

    f32 = mybir.dt.float32
    bf16 = mybir.dt.bfloat16
    OP = mybir.AluOpType
    AF = mybir.ActivationFunctionType

    nc = bacc.Bacc("TRN2", target_bir_lowering=False, debug=False,
                   num_devices=N_CORES)

    xs_d = nc.dram_tensor("xs", [128, NBLK, BL], f32, kind="ExternalInput").ap()
    wd_d = nc.dram_tensor("wd", [128, OL, NBLK, 16], f32,
                          kind="ExternalInput").ap()
    kcs_d = nc.dram_tensor("kcs", [OL, BL, 1], f32, kind="ExternalInput").ap()
    v_d = nc.dram_tensor("v", [OL, BL, 16], f32, kind="ExternalOutput").ap()

    with tile.TileContext(nc) as tc:
        from contextlib import ExitStack
        ctx = ExitStack()
        u_pool = ctx.enter_context(tc.tile_pool(name="u", bufs=3))
        wbd_pool = ctx.enter_context(tc.tile_pool(name="wbd", bufs=1))
        wd_pool = ctx.enter_context(tc.tile_pool(name="wd", bufs=1))
        xs_pool = ctx.enter_context(tc.tile_pool(name="xs", bufs=2))
        e_pool = ctx.enter_context(tc.tile_pool(name="e", bufs=2))
        st_pool = ctx.enter_context(tc.tile_pool(name="st", bufs=2))
        scr_pool = ctx.enter_context(tc.tile_pool(name="scr", bufs=1))
        pu_pool = ctx.enter_context(tc.tile_pool(name="pu", bufs=2, space="PSUM"))
        ps1_pool = ctx.enter_context(tc.tile_pool(name="ps1", bufs=1, space="PSUM"))

        u32 = mybir.dt.uint32
        shift_t = st_pool.tile([BL, 1], f32, name="shift")
        nc.vector.memset(shift_t[:], SHIFT)
        ones_t = st_pool.tile([BL, 16], f32, name="ones")
        nc.vector.memset(ones_t[:], 1.0)
        magic_u = st_pool.tile([BL, 1], u32, name="magic")
        nc.vector.memset(magic_u[:], 0x5f3759df)

        # block-diagonal moving operand, zeroed once; diagonal blocks are
        # overwritten per o (same positions every o, zeros stay zero).
        wbd_t = wbd_pool.tile([128, NBLK, 256], f32, name="wbd")
        half = NBLK * 256 // 2
        nc.vector.memset(wbd_t[:].rearrange("p a b -> p (a b)")[:, :half], 0.0)
        nc.scalar.memzero(wbd_t[:].rearrange("p a b -> p (a b)")[:, half:])

        # persistent scratch for STT main outputs (value never read)
        scr_v = scr_pool.tile([BL, N], bf16, name="scrv")

        for oi in range(reps * OL):
            o = oi % OL
            # dense W for this o (feeds the diag build + iter-1 mean matmul)
            wd_t = wd_pool.tile([128, NBLK, 16], f32, name=f"wd{oi}", tag="wd")
            nc.gpsimd.dma_start(wd_t[:], wd_d[:, o])

            # ---- build block-diagonal W (SBUF->SBUF DMA; engine ops can't
            # start at unaligned partitions) ----
            if oi == 0:
                bqueues = [nc.sync, nc.gpsimd, nc.scalar]
            else:
                bqueues = [nc.sync, nc.gpsimd]
            for nn in range(16):
                dst = wbd_t[nn * 8:(nn + 1) * 8, :, nn * 16:(nn + 1) * 16]
                src = wd_t[nn * 8:(nn + 1) * 8, :, :]
                bqueues[nn % len(bqueues)].dma_start(dst, src)

            kcs_t = st_pool.tile([BL, 1], f32, name=f"kcs{oi}", tag="kcs")
            nc.gpsimd.dma_start(kcs_t[:], kcs_d[o])

            # u in two d-halves (separate tiles from a 3-deep pool so the
            # next chunk's evacuation overlaps this chunk's routing)
            uh = [u_pool.tile([128, 8, N], f32, name=f"u{oi}_{h}", tag="u")
                  for h in range(2)]
            ps1 = ps1_pool.tile([128, 16], f32, name=f"s1p{oi}", tag="s1p")

            # ---- phase 1: u_hat matmuls + PSUM->SBUF evacuation ----
            # (priority-boosted ~one chunk so the scheduler interleaves this
            # with the previous chunk's routing instead of queueing behind it)
            ctx_p = tc.high_priority(offset=800) if oi > 0 else None
            if ctx_p is not None:
                ctx_p.__enter__()
            for g in range(NGRP):
                pu = pu_pool.tile([128, GB, 256], f32, name=f"pu{oi}_{g}",
                                  tag="pu")
                xs_t = xs_pool.tile([128, GB, BL], f32, name=f"x{oi}_{g}",
                                    tag="xs")
                nc.gpsimd.dma_start(xs_t[:], xs_d[:, g * GB:(g + 1) * GB, :])
                for j in range(GB):
                    blk = g * GB + j
                    nc.tensor.matmul(pu[:, j, :], lhsT=xs_t[:, j, :],
                                     rhs=wbd_t[:, blk, :],
                                     start=True, stop=True,
                                     skip_group_check=True)
                    nc.tensor.matmul(ps1[:], lhsT=xs_t[:, j, :],
                                     rhs=wd_t[:, blk, :],
                                     start=(blk == 0), stop=(blk == NBLK - 1),
                                     skip_group_check=True)
                # evacuate: [128, (j, nn16, d16)] -> u[128, d, g*96+(j,nn)]
                for h in range(2):
                    src = pu[:].rearrange(
                        "p j (nn d) -> p d j nn", nn=16, d=16
                    )[:, h * 8:(h + 1) * 8]
                    dst = uh[h][:, :, g * 96:(g + 1) * 96].rearrange(
                        "p d (j nn) -> p d j nn", j=GB, nn=16)
                    if (2 * g + h) % 12 < 7:
                        nc.scalar.copy(dst, src)
                    else:
                        nc.vector.tensor_copy(dst, src)

            # ---- iteration 1 (uniform softmax): s = mean_n u ----
            s_t = st_pool.tile([BL, 16], f32, name=f"s{oi}", tag="s")
            nc.scalar.mul(s_t[:], ps1[:], 1.0 / N)

            sq2 = st_pool.tile([BL, 16], f32, name=f"sq2_{oi}", tag="sq2")
            T_t = st_pool.tile([BL, 1], f32, name=f"T{oi}", tag="T")
            vout = st_pool.tile([BL, 16], f32, name=f"vo{oi}", tag="vo")

            def squash_T(tag, accum_into_T, want_T=True):
                # f = sqrt(sn)/(1+sn); vout = f*s; Tdelta = sum_d vout
                # sqrt via fast-inverse-sqrt (bit trick + 3 Newton steps),
                # all on DVE: the ACT Ln/Exp route forces a ~1.3-2.7us table
                # reload per call (Ln and Exp land in different table sets).
                sn = st_pool.tile([BL, 1], f32, name=f"sn{tag}", tag="sn")
                nc.vector.scalar_tensor_tensor(
                    sq2[:], s_t[:], 1.0, s_t[:], op0=OP.mult, op1=OP.mult,
                    accum_out=sn[:])
                y_t = st_pool.tile([BL, 1], f32, name=f"y{tag}", tag="y")
                t2 = st_pool.tile([BL, 1], f32, name=f"t2{tag}", tag="t2")
                # y0 bits = MAGIC - (sn_bits >> 1)
                nc.vector.tensor_scalar(
                    t2[:].bitcast(u32), sn[:].bitcast(u32), 1, None,
                    op0=OP.logical_shift_right)
                nc.vector.tensor_tensor(
                    y_t[:].bitcast(u32), magic_u[:], t2[:].bitcast(u32),
                    op=OP.subtract)
                for _ in range(2):
                    # y <- y * (1.5 - 0.5*sn*y^2), 3 fused DVE ops
                    nc.vector.tensor_mul(t2[:], y_t[:], y_t[:])
                    nc.vector.scalar_tensor_tensor(
                        t2[:], sn[:], -0.5, t2[:], op0=OP.mult, op1=OP.mult)
                    nc.vector.scalar_tensor_tensor(
                        y_t[:], t2[:], 1.5, y_t[:], op0=OP.add, op1=OP.mult)
                snp = st_pool.tile([BL, 1], f32, name=f"sp{tag}", tag="snp")
                nc.vector.tensor_scalar_add(snp[:], sn[:], 1.0)
                rsn = st_pool.tile([BL, 1], f32, name=f"rs{tag}", tag="rsn")
                nc.vector.reciprocal(rsn[:], snp[:])
                f_t = st_pool.tile([BL, 1], f32, name=f"f{tag}", tag="f")
                nc.vector.scalar_tensor_tensor(
                    f_t[:], sn[:], rsn[:], y_t[:], op0=OP.mult, op1=OP.mult)
                if not want_T:
                    nc.vector.scalar_tensor_tensor(
                        vout[:], s_t[:], f_t[:], ones_t[:],
                        op0=OP.mult, op1=OP.mult)
                    return
                dT = st_pool.tile([BL, 1], f32, name=f"dT{tag}", tag="dT")
                nc.vector.scalar_tensor_tensor(
                    vout[:], s_t[:], f_t[:], ones_t[:],
                    op0=OP.mult, op1=OP.mult, accum_out=dT[:])
                if accum_into_T:
                    nc.vector.tensor_add(T_t[:], T_t[:], dT[:])
                else:
                    nc.vector.tensor_copy(T_t[:], dT[:])

            squash_T(f"{oi}i1", accum_into_T=False)

            den_t = st_pool.tile([BL, 16], f32, name=f"den{oi}", tag="den")
            num_t = st_pool.tile([BL, 16], f32, name=f"num{oi}", tag="num")

            for it in (2, 3):
                absT = st_pool.tile([BL, 1], f32, name=f"aT{oi}_{it}", tag="aT")
                nc.vector.scalar_tensor_tensor(
                    absT[:], T_t[:], -1.0, T_t[:], op0=OP.mult, op1=OP.max)
                negm = st_pool.tile([BL, 1], f32, name=f"nm{oi}_{it}", tag="nm")
                nc.vector.scalar_tensor_tensor(
                    negm[:], absT[:], kcs_t[:], shift_t[:],
                    op0=OP.mult, op1=OP.add)
                for d in range(D):
                    u_sl = uh[d // 8][:, d % 8, :]
                    e_t = e_pool.tile([128, N], f32, name=f"e{oi}_{it}_{d}",
                                      tag="e")
                    nc.scalar.activation(e_t[:], u_sl, AF.Exp,
                                         bias=negm[:], scale=T_t[:],
                                         accum_out=den_t[:, d:d + 1])
                    nc.vector.scalar_tensor_tensor(
                        out=scr_v[:], in0=e_t[:], scalar=1.0,
                        in1=u_sl, op0=OP.mult, op1=OP.mult,
                        accum_out=num_t[:, d:d + 1])
                rden = st_pool.tile([BL, 16], f32, name=f"rd{oi}_{it}", tag="rd")
                nc.vector.reciprocal(rden[:], den_t[:])
                nc.vector.tensor_mul(s_t[:], num_t[:], rden[:])
                squash_T(f"{oi}i{it}", accum_into_T=(it == 2), want_T=(it == 2))
                if it == 2 and ctx_p is not None:
                    ctx_p.__exit__(None, None, None)
                    ctx_p = None

            nc.gpsimd.dma_start(v_d[o], vout[:])
        ctx.close()

    nc.compile()
    return nc


def _pack_w(W):
    # W: [N, O, D, I] fp32 -> per o-group: dense pack [(nn,i), o, blk, d]
    # plus the Cauchy-Schwarz bound factors.
    packs = []
    for og in range(OG):
        Wl = W[:, og * OL:(og + 1) * OL]                     # [N, 5, D, I]
        Wt = Wl.reshape(NBLK, 16, OL, D, I)                  # [blk, nn, o, d, i]
        wd = np.ascontiguousarray(
            Wt.transpose(1, 4, 2, 0, 3).reshape(128, OL, NBLK, 16)
        ).astype(np.float32)
        wmax = np.sqrt((Wl ** 2).sum(-1)).max(axis=(0, 2))   # [5]
        packs.append((wd, wmax.astype(np.float32)))
    return packs


def _get_state(W):
    key = (W.shape, hash(W[::131, 0, 0, 0].tobytes()))
    st = _STATE.get("st")
    if st is not None and st["key"] == key:
        return st
    nc = _STATE.get("nc")
    if nc is None:
        nc = _build_bass()
        _STATE["nc"] = nc
    st = {"key": key, "nc": nc, "wpacks": _pack_w(W)}
    _STATE["st"] = st
    return st


def _core_inputs(st, x, xnorm):
    # returns in_maps for the 8 cores; core index = og * BG + bg
    in_maps = []
    for og in range(OG):
        wd, wmax = st["wpacks"][og]
        for bg in range(BG):
            b0 = bg * BL
            xc = x[b0:b0 + BL]                               # [128, N, I]
            xs = np.ascontiguousarray(
                xc.transpose(1, 2, 0).reshape(NBLK, 128, BL)
                .transpose(1, 0, 2))                         # [p, blk, b]
            kcs = -(wmax[:, None] * xnorm[None, b0:b0 + BL]) * 1.001  # [5,128]
            in_maps.append({
                "xs": xs, "wd": wd,
                "kcs": np.ascontiguousarray(
                    kcs[:, :, None].astype(np.float32)),
            })
    return in_maps


class _Runner:
    """Cached PJRT executor for the compiled Bass program (modeled on
    bass2jax.run_bass_via_pjrt, but holding the jitted callable so repeat
    calls don't re-trace)."""

    def __init__(self, nc):
        import jax
        import numpy as _np
        from jax.sharding import Mesh, PartitionSpec
        from jax.experimental.shard_map import shard_map
        from concourse import mybir
        from concourse.bass2jax import (_bass_exec_p, install_neuronx_cc_hook,
                                        partition_id_tensor)

        install_neuronx_cc_hook()
        self.jax = jax
        part_name = (nc.partition_id_tensor.name
                     if nc.partition_id_tensor else None)
        in_names, out_names, out_avals, zero_outs = [], [], [], []
        for alloc in nc.m.functions[0].allocations:
            if not isinstance(alloc, mybir.MemoryLocationSet):
                continue
            name = alloc.memorylocations[0].name
            if alloc.kind == "ExternalInput":
                if name == part_name:
                    continue
                in_names.append(name)
            elif alloc.kind == "ExternalOutput":
                shape = tuple(alloc.tensor_shape)
                dtype = mybir.dt.np(alloc.dtype)
                out_names.append(name)
                out_avals.append(jax.core.ShapedArray(shape, dtype))
                zero_outs.append(_np.zeros(shape, dtype))
        self.in_names, self.out_names = in_names, out_names
        self.out_avals, self.zero_outs = out_avals, zero_outs
        n_params, n_outs = len(in_names), len(out_names)
        all_names = in_names + out_names
        if part_name is not None:
            all_names = all_names + [part_name]

        def _body(*args):
            ins = list(args[:n_params])
            outs = list(args[n_params:])
            pid = [partition_id_tensor()] if part_name is not None else []
            outs = list(_bass_exec_p.bind(
                *ins, *outs, *pid,
                out_avals=tuple(out_avals),
                in_names=tuple(all_names),
                out_names=tuple(out_names),
                lowering_input_output_aliases=(),
                sim_require_finite=True, sim_require_nnan=True,
                nc=nc))
            return tuple(outs)

        devices = jax.devices()[:N_CORES]
        self.mesh = Mesh(_np.asarray(devices), ("core",))
        in_specs = (PartitionSpec("core"),) * (n_params + n_outs)
        out_specs = (PartitionSpec("core"),) * n_outs
        self.fn = jax.jit(
            shard_map(_body, mesh=self.mesh,
                      in_specs=in_specs, out_specs=out_specs,
                      check_rep=False),
            donate_argnums=tuple(range(n_params, n_params + n_outs)),
            keep_unused=True)

    def concat_inputs(self, in_maps):
        import numpy as _np
        return [_np.concatenate([m[name] for m in in_maps], axis=0)
                for name in self.in_names]

    def zeros(self):
        import numpy as _np
        return [_np.zeros((N_CORES * z.shape[0], *z.shape[1:]), z.dtype)
                for z in self.zero_outs]

    def __call__(self, concat_in):
        return self.fn(*concat_in, *self.zeros())


def _run(st, in_maps):
    runner = st.get("runner")
    if runner is None:
        runner = _Runner(st["nc"])
        st["runner"] = runner
    out_arrs = runner(runner.concat_inputs(in_maps))
    av = runner.out_avals[0]
    import numpy as _np
    v_all = _np.asarray(out_arrs[0]).reshape(N_CORES, *av.shape)
    return v_all


def kernel(x: np.ndarray, W: np.ndarray) -> np.ndarray:
    x = np.ascontiguousarray(x, dtype=np.float32)
    W = np.ascontiguousarray(W, dtype=np.float32)
    st = _get_state(W)
    xnorm = np.sqrt((x ** 2).sum(-1)).max(axis=1).astype(np.float32)  # [B]
    in_maps = _core_inputs(st, x, xnorm)
    v_all = _run(st, in_maps)
    out = np.empty((B, O, D), np.float32)
    ci = 0
    for og in range(OG):
        for bg in range(BG):
            v = v_all[ci]                                     # [5, 128, 16]
            out[bg * BL:(bg + 1) * BL, og * OL:(og + 1) * OL, :] = \
                v.transpose(1, 0, 2)
            ci += 1
    return out


def hw_exec_ns(x: np.ndarray, W: np.ndarray, reps: int = 4,
               chain_len: int = 16) -> int:
    """Device execution time per kernel body, measured by differencing two
    NEFFs: one with the body once, one with it repeated `reps` times.  Both
    are dispatched as identical async call-chains, so host/axon dispatch
    overhead cancels exactly in the subtraction."""
    import time
    import jax
    x = np.ascontiguousarray(x, dtype=np.float32)
    W = np.ascontiguousarray(W, dtype=np.float32)
    st = _get_state(W)
    xnorm = np.sqrt((x ** 2).sum(-1)).max(axis=1).astype(np.float32)
    in_maps = _core_inputs(st, x, xnorm)
    runner = st.get("runner")
    if runner is None:
        runner = _Runner(st["nc"])
        st["runner"] = runner
    nc_r = _STATE.get("nc_rep")
    if nc_r is None:
        nc_r = _build_bass(reps=reps)
        _STATE["nc_rep"] = nc_r
    runner_r = st.get("runner_rep")
    if runner_r is None:
        runner_r = _Runner(nc_r)
        st["runner_rep"] = runner_r

    from jax.sharding import NamedSharding, PartitionSpec
    sh = NamedSharding(runner.mesh, PartitionSpec("core"))
    dev_in = [jax.device_put(a, sh) for a in runner.concat_inputs(in_maps)]

    def chain(fn, n):
        outs = [jax.device_put(z, sh) for z in runner.zeros()]
        jax.block_until_ready(outs)
        t0 = time.perf_counter()
        for _ in range(n):
            outs = fn(*dev_in, *outs)
        jax.block_until_ready(outs)
        return time.perf_counter() - t0

    chain(runner.fn, 2)       # warm + compile
    chain(runner_r.fn, 2)
    t1 = min(chain(runner.fn, chain_len) for _ in range(6))
    tr = min(chain(runner_r.fn, chain_len) for _ in range(6))
    return int((tr - t1) / (chain_len * (reps - 1)) * 1e9)
